# revision 1
# baseline (speedup 1.0000x reference)
"""DecNFM rating-loss forward on 8 Trainium2 NeuronCores.

Strategy (data-parallel, v3):
  - Shard the batch (16384) across 8 cores -> 2048 rows/core.
  - Per core: 48 indirect-DMA gathers of bf16 embedding rows issued
    up-front back-to-back (row-bound on the Pool engine; Pool does
    nothing else).
  - FM cross terms in 7 plain DVE tensor_tensor ops per group (8 groups
    of 256 rows; plain TT runs 2 elem/cycle, scalar_tensor_tensor only 1):
      fm = r + m,  r = ue*(ie + p), p = (t+ce)*cs, m = t*ce, t = ue+ie
    (same algebra as the reference FM with the ~5e-9 constant dropped).
  - Transpose fm into [D, B] via regular matmuls against a 2^15-scaled
    identity (fp32 PSUM) — the fp8 pre-scale rides the identity for free.
  - ACT converts PSUM to fp8 e4m3; MLP in fp8 with DoubleRow perf mode
    (256-deep contraction per instruction); weights pre-scaled 2^12,
    activations rescaled via ACT Relu scale; the exact power-of-2 scale
    chain is undone in the final sigmoid (scale 2^-30).
  - L2 reg via fp32 row-norms gathered in slots [512:514] of each row.
  - Host combines 8+3 partial sums per core into the scalar loss.
"""

from contextlib import ExitStack

import ml_dtypes
import numpy as np

import concourse.bass as bass
import concourse.bass_isa as bass_isa
import concourse.tile as tile
from concourse import bacc, mybir
from concourse.bass_utils import run_bass_kernel_spmd
from concourse.masks import make_identity

BF = ml_dtypes.bfloat16
F8 = ml_dtypes.float8_e4m3
F32 = np.float32
L2RG = 1e-4

NCORES = 8
B = 16384
BL = B // NCORES      # 2048 batch rows per core
D = 512
RW = 516              # row width: 512 emb + 2 norm slots + 2 pad
KCH = D // 128        # 4 contraction chunks
NCH = BL // 128       # 16 chunks of 128 rows
# uneven groups: big groups overlap the gather stream, small final groups
# shorten the post-gather serial tail
GROUPS = [(0, 2), (2, 2), (4, 2), (6, 2), (8, 2), (10, 2), (12, 2),
          (14, 1), (15, 1)]  # (chunk_start, n_chunks)
G = len(GROUPS)
JPG = 2               # max chunks per group (tile sizing)
GB = 128 * JPG        # max rows per group (tile sizing)

U_ROWS = 200000
I_ROWS = 100000
C_ROWS = 2000

S15 = float(2.0 ** 15)

AD = mybir.AluOpType
AF = mybir.ActivationFunctionType
PM = mybir.MatmulPerfMode
DT = mybir.dt


def _build():
    nc = bacc.Bacc("TRN2", target_bir_lowering=False, debug=False)

    d_uw = nc.dram_tensor("uaug", [U_ROWS, RW], DT.bfloat16, kind="ExternalInput")
    d_iw = nc.dram_tensor("iaug", [I_ROWS, RW], DT.bfloat16, kind="ExternalInput")
    d_cw = nc.dram_tensor("caug", [C_ROWS, RW], DT.bfloat16, kind="ExternalInput")
    d_ui = nc.dram_tensor("uidx", [128, NCH], DT.int32, kind="ExternalInput")
    d_ii = nc.dram_tensor("iidx", [128, NCH], DT.int32, kind="ExternalInput")
    d_ci = nc.dram_tensor("cidx", [128, NCH], DT.int32, kind="ExternalInput")
    d_rd = nc.dram_tensor("rdelta", [1, BL], DT.float32, kind="ExternalInput")
    d_cs = nc.dram_tensor("csrow", [1, JPG * D], DT.bfloat16, kind="ExternalInput")
    # fp8 DoubleRow weights: [kk, 128, 2, 512] with [p, i, m] = w[(2kk+i)*128+p, m]
    d_w1 = nc.dram_tensor("w1d", [2, 128, 2, D], DT.float8e4, kind="ExternalInput")
    d_w2 = nc.dram_tensor("w2d", [2, 128, 2, D], DT.float8e4, kind="ExternalInput")
    d_w3 = nc.dram_tensor("w3c", [128, KCH], DT.float8e4, kind="ExternalInput")
    d_b3 = nc.dram_tensor("b3s", [1, 1], DT.float32, kind="ExternalInput")
    d_out = nc.dram_tensor("out", [1, G + 3], DT.float32, kind="ExternalOutput")

    with tile.TileContext(nc) as tc, ExitStack() as ctx:
        per = ctx.enter_context(tc.tile_pool(name="per", bufs=1))
        strm = ctx.enter_context(tc.tile_pool(name="strm", bufs=2))
        psT = ctx.enter_context(tc.tile_pool(name="psT", bufs=2, space="PSUM"))
        psmm = ctx.enter_context(tc.tile_pool(name="psmm", bufs=2, space="PSUM"))
        psl = ctx.enter_context(tc.tile_pool(name="psl", bufs=2, space="PSUM"))

        # ---- index tiles first so gathers can start immediately ----
        uidx = per.tile([128, NCH], DT.int32)
        iidx = per.tile([128, NCH], DT.int32)
        cidx = per.tile([128, NCH], DT.int32)
        nc.sync.dma_start(uidx[:], d_ui.ap())
        nc.sync.dma_start(iidx[:], d_ii.ap())
        nc.sync.dma_start(cidx[:], d_ci.ap())

        # ---- all 48 gathers up-front, Pool does nothing else ----
        gt = {}
        for g, (c0, jpg) in enumerate(GROUPS):
            ga = strm.tile([128, JPG, RW], DT.bfloat16, tag="ga", name=f"ga{g}", bufs=G)
            gb = strm.tile([128, JPG, RW], DT.bfloat16, tag="gb", name=f"gb{g}", bufs=G)
            gc = strm.tile([128, JPG, RW], DT.bfloat16, tag="gc", name=f"gc{g}", bufs=G)
            gt[g] = (ga, gb, gc)
            for t, tab, idxt in ((ga, d_uw, uidx), (gb, d_iw, iidx), (gc, d_cw, cidx)):
                for j in range(jpg):
                    c = c0 + j
                    nc.gpsimd.indirect_dma_start(
                        out=t[:, j, :], out_offset=None, in_=tab.ap()[:, :],
                        in_offset=bass.IndirectOffsetOnAxis(ap=idxt[:, c:c + 1], axis=0),
                    )

        # ---- remaining constants (behind the gathers on the sync queue) ----
        rdelta = per.tile([1, BL], DT.float32)
        nc.sync.dma_start(rdelta[:], d_rd.ap())
        w1t = [per.tile([128, 2, D], DT.float8e4, name=f"w1_{k}") for k in range(2)]
        w2t = [per.tile([128, 2, D], DT.float8e4, name=f"w2_{k}") for k in range(2)]
        for k in range(2):
            nc.sync.dma_start(w1t[k][:], d_w1.ap()[k])
            nc.sync.dma_start(w2t[k][:], d_w2.ap()[k])
        w3t = per.tile([128, KCH], DT.float8e4)
        nc.sync.dma_start(w3t[:], d_w3.ap())
        b3t = per.tile([1, 1], DT.float32)
        nc.sync.dma_start(b3t[:], d_b3.ap())
        csb = per.tile([128, JPG * D], DT.bfloat16)
        nc.sync.dma_start(csb[:], d_cs.ap()[:, :].to_broadcast([128, JPG * D]))

        ident = per.tile([128, 128], DT.bfloat16)
        make_identity(nc, ident[:])
        identS = per.tile([128, 128], DT.bfloat16)
        nc.scalar.mul(identS[:], ident[:], S15)

        fmT = per.tile([128, KCH, BL], DT.float8e4)
        h1T = per.tile([128, KCH, BL], DT.float8e4)
        h2T = per.tile([128, KCH, BL], DT.float8e4)
        zbias = per.tile([128, 1], DT.float32)
        nc.vector.memset(zbias[:], 0.0)
        ssec = per.tile([1, G], DT.float32)
        racc12 = per.tile([128, 3 * G], DT.float32)
        csbv = csb[:].rearrange("p (a b) -> p a b", a=JPG)

        for g, (c0, jpg) in enumerate(GROUPS):
            gb_rows = 128 * jpg
            gsp = slice(c0 * 128, c0 * 128 + gb_rows)
            ga, gb, gc = gt[g]
            ue = ga[:, :jpg, 0:D]
            ie = gb[:, :jpg, 0:D]
            ce = gc[:, :jpg, 0:D]

            t = strm.tile([128, JPG, D], DT.bfloat16, tag="t", name=f"t{g}")
            p = strm.tile([128, JPG, D], DT.bfloat16, tag="p", name=f"p{g}")
            q = strm.tile([128, JPG, D], DT.bfloat16, tag="q", name=f"q{g}")
            r = strm.tile([128, JPG, D], DT.bfloat16, tag="r", name=f"r{g}")
            m = strm.tile([128, JPG, D], DT.bfloat16, tag="m", name=f"m{g}")
            fm = strm.tile([128, JPG, D], DT.bfloat16, tag="fm", name=f"fm{g}")

            # fm via 7 plain TT ops (all 2 elem/cycle)
            tv, pv, qv, rv, mv, fmv = (x[:, :jpg, :] for x in (t, p, q, r, m, fm))
            csj = csbv[:, :jpg, :]
            nc.vector.tensor_tensor(tv, ue, ie, AD.add)
            nc.vector.tensor_tensor(pv, tv, ce, AD.add)
            nc.vector.tensor_tensor(pv, pv, csj, AD.mult)
            nc.vector.tensor_tensor(qv, ie, pv, AD.add)
            nc.vector.tensor_tensor(rv, ue, qv, AD.mult)
            nc.vector.tensor_tensor(mv, tv, ce, AD.mult)
            nc.vector.tensor_tensor(fmv, rv, mv, AD.add)

            # reg partials: fp32 norms bitcast at slots [512:514]
            for t_i, tt in enumerate((ga, gb, gc)):
                nrm = tt[:, :jpg, D:D + 2].bitcast(DT.float32)
                nc.vector.tensor_reduce(
                    out=racc12[:, t_i * G + g: t_i * G + g + 1], in_=nrm,
                    axis=mybir.AxisListType.XY, op=AD.add,
                )

            # transpose fm -> psT fp32 scaled 2^15 (regular matmul vs 2^15*I)
            pt = psT.tile([128, KCH, GB], DT.float32, space="PSUM",
                          tag="psT", name=f"psT{g}")
            for dk in range(KCH):
                for j in range(jpg):
                    nc.tensor.matmul(
                        out=pt[:, dk, j * 128:(j + 1) * 128],
                        lhsT=fm[:, j, dk * 128:(dk + 1) * 128],
                        rhs=identS[:],
                        start=True, stop=True,
                    )
            nc.scalar.activation(fmT[:, :, gsp], pt[:, :, :gb_rows], AF.Copy)

            # MLP: fp8 DoubleRow, 256-deep contraction per matmul
            for li, (wt, inT, outT, scl) in enumerate((
                (w1t, fmT, h1T, 2.0 ** -10),
                (w2t, h1T, h2T, 2.0 ** -11),
            )):
                for mp in range(KCH // 2):
                    pm = psmm.tile([128, 2, GB], DT.float32, space="PSUM",
                                   tag="psmm", name=f"ps{li}_{g}_{mp}")
                    for mh in range(2):
                        mb = mp * 2 + mh
                        for kk in range(2):
                            nc.tensor.matmul(
                                out=pm[:, mh, :gb_rows],
                                lhsT=wt[kk][:, :, mb * 128:(mb + 1) * 128],
                                rhs=inT[:, 2 * kk:2 * kk + 2, gsp],
                                start=(kk == 0), stop=(kk == 1),
                                perf_mode=PM.DoubleRow,
                            )
                    nc.scalar.activation(
                        outT[:, mp * 2:mp * 2 + 2, gsp], pm[:, :, :gb_rows],
                        AF.Relu, bias=zbias[:, :1], scale=scl,
                    )

            # logits (plain fp8 matmul) + sigmoid + sse partial
            pl = psl.tile([1, GB], DT.float32, space="PSUM", tag="psl", name=f"psl{g}")
            for k in range(KCH):
                nc.tensor.matmul(
                    out=pl[:, :gb_rows], lhsT=w3t[:, k:k + 1], rhs=h2T[:, k, gsp],
                    start=(k == 0), stop=(k == KCH - 1),
                )
            sig = strm.tile([1, GB], DT.float32, tag="sig", name=f"sig{g}")
            nc.scalar.activation(sig[:, :gb_rows], pl[:, :gb_rows], AF.Sigmoid,
                                 bias=b3t[:1, :1], scale=2.0 ** -30)
            dd = strm.tile([1, GB], DT.float32, tag="dd", name=f"dd{g}")
            nc.vector.scalar_tensor_tensor(
                out=dd[:, :gb_rows], in0=sig[:, :gb_rows], scalar=4.0,
                in1=rdelta[:, gsp],
                op0=AD.mult, op1=AD.subtract,
            )
            dsq = strm.tile([1, GB], DT.float32, tag="dsq", name=f"dsq{g}")
            nc.vector.scalar_tensor_tensor(
                out=dsq[:, :gb_rows], in0=dd[:, :gb_rows], scalar=1.0,
                in1=dd[:, :gb_rows],
                op0=AD.mult, op1=AD.mult, accum_out=ssec[:, g:g + 1],
            )

        # ---- reg partials ----
        racc = per.tile([128, 3], DT.float32)
        for t_i in range(3):
            nc.vector.tensor_reduce(
                out=racc[:, t_i:t_i + 1], in_=racc12[:, t_i * G:(t_i + 1) * G],
                axis=mybir.AxisListType.X, op=AD.add,
            )
        # partition reduce via ones-vector fp32 matmul (partition_all_reduce
        # forces a ~7us Pool DMA drain before its Q7 ucode runs)
        ones = per.tile([128, 1], DT.float32)
        nc.vector.memset(ones[:], 1.0)
        prg = psl.tile([1, 3], DT.float32, space="PSUM", tag="psl", name="prg")
        nc.tensor.matmul(out=prg[:], lhsT=ones[:], rhs=racc[:],
                         start=True, stop=True)
        rall = per.tile([1, 3], DT.float32)
        nc.scalar.activation(rall[:], prg[:], AF.Copy)
        nc.sync.dma_start(d_out.ap()[:, 0:G], ssec[:])
        nc.sync.dma_start(d_out.ap()[:, G:G + 3], rall[:1, :3])

    nc.compile()
    return nc


_CACHE: dict = {}


def _augment(w: np.ndarray) -> np.ndarray:
    """[V, D] fp32 -> [V, RW] bf16 rows: emb | fp32 rownorm bitcast | pad."""
    v = w.shape[0]
    norm = np.square(w, dtype=F32).sum(axis=1, dtype=np.float64).astype(F32)
    aug = np.zeros((v, RW), dtype=np.uint16)
    aug[:, :D] = w.astype(BF).view(np.uint16)
    aug[:, D:D + 2] = norm.view(np.uint16).reshape(v, 2)
    return aug.view(BF)


def _dr_weights(w: np.ndarray, scale: float) -> np.ndarray:
    """[512, 512] fp32 -> DoubleRow lhsT [kk=2, 128, i=2, 512] fp8."""
    ws = (w * scale).astype(F8)
    out = np.zeros((2, 128, 2, D), dtype=F8)
    for kk in range(2):
        for i in range(2):
            out[kk, :, i, :] = ws[(2 * kk + i) * 128:(2 * kk + i + 1) * 128, :]
    return out


def _prep(inputs):
    user = np.ascontiguousarray(np.asarray(inputs["user"]).astype(np.int64))
    item = np.ascontiguousarray(np.asarray(inputs["item"]).astype(np.int64))
    cate = np.ascontiguousarray(np.asarray(inputs["cate"]).astype(np.int64))
    rate = np.asarray(inputs["rate"], dtype=F32)
    uw = np.asarray(inputs["user_w"], dtype=F32)
    iw = np.asarray(inputs["item_w"], dtype=F32)
    cw = np.asarray(inputs["cate_w"], dtype=F32)
    prior = np.asarray(inputs["cate_prior"], dtype=F32)
    w1 = np.asarray(inputs["w1"], dtype=F32)
    w2 = np.asarray(inputs["w2"], dtype=F32)
    w3 = np.asarray(inputs["w3"], dtype=F32)
    b3 = np.asarray(inputs["b3"], dtype=F32)

    wc = cw.astype(np.float64) * prior.astype(np.float64)[:, None]
    cs = wc.sum(axis=0).astype(F32)

    shared = {
        "uaug": _augment(uw),
        "iaug": _augment(iw),
        "caug": _augment(cw),
        "csrow": np.ascontiguousarray(np.tile(cs.astype(BF), JPG)[None, :]),
        "w1d": _dr_weights(w1, 2.0 ** 12),
        "w2d": _dr_weights(w2, 2.0 ** 12),
        "w3c": np.ascontiguousarray(
            (w3[:, 0] * 2.0 ** 12).astype(F8).reshape(KCH, 128).T),
        "b3s": b3.reshape(1, 1),
    }

    def colmajor(ids):
        return np.ascontiguousarray(ids.reshape(NCH, 128).T.astype(np.int32))

    in_maps = []
    for c in range(NCORES):
        sl = slice(c * BL, (c + 1) * BL)
        mm = dict(shared)
        mm["uidx"] = colmajor(user[sl])
        mm["iidx"] = colmajor(item[sl])
        mm["cidx"] = colmajor(cate[sl])
        mm["rdelta"] = np.ascontiguousarray((rate[sl] - 1.0)[None, :])
        in_maps.append(mm)
    return in_maps


def kernel(**inputs) -> np.ndarray:
    in_maps = _prep(inputs)
    if "nc" not in _CACHE:
        _CACHE["nc"] = _build()
    res = run_bass_kernel_spmd(_CACHE["nc"], in_maps, list(range(NCORES)))
    sse = 0.0
    reg = 0.0
    for c in range(NCORES):
        out = np.asarray(res.results[c]["out"], dtype=np.float64)[0]
        sse += out[0:G].sum()
        reg += out[G:G + 3].sum()
    loss = sse / B + L2RG * (0.5 * reg) / B
    return np.array(loss, dtype=F32)

